# revision 22
# baseline (speedup 1.0000x reference)
"""JKConv (8-layer GCN + jumping-knowledge max pool) Bass kernel for 8 trn2 cores.

Distribution (per the node-partitioning hint): destination nodes are sharded
contiguously across the 8 cores (6250 each). Per layer:
  1. every core holds the full gather table h (bf16, [N,128]) in its DRAM;
  2. per-edge source rows are fetched with indirect DMA (gather), 128 edges
     per block, grouped so that <=128 destination nodes ("a group") are
     aggregated by a chain of TensorE matmuls accumulating into one PSUM tile:
        aggT[fi, slot] += msg_b[e, fi].T @ S'_b[e, slot]
     where S'_b[e, slot] = norm_e * [dst_e == slot] is built on-chip on DVE
     from (slot, norm) per-edge metadata with an iota/is_equal trick;
  3. z = W.T @ aggT (+bias), ELU on ACT/DVE, JK running max (all in the
     feature-major orientation), transpose back and write the local h block;
  4. an AllGather collective rebuilds the full table for the next layer.
Host side does graph preprocessing (self loops, symmetric norm, dst sort,
degree-balanced grouping permutation) and the inverse permutation on output.
"""

import numpy as np

N_NODES = 50000
D = 128
K_LAYERS = 8
N_CORES = 8


# ---------------------------------------------------------------- host prep

def _snake_groups(groups, gsizes):
    """Sequence of group ids so that sorted-by-degree nodes spread evenly."""
    caps = list(gsizes)
    seq = []
    r = 0
    while len(seq) < sum(gsizes):
        elig = [g for g in range(groups) if caps[g] > r]
        if not elig:
            r += 1
            continue
        if r % 2:
            elig = elig[::-1]
        seq.extend(elig)
        r += 1
    return np.asarray(seq[: sum(gsizes)], dtype=np.int64)


def prepare_graph(edge_index, n, ncore):
    """Preprocess edges -> per-core block tensors.

    Returns dict with per-core [128, B] arrays (gidx/rel/nrm), node order
    arrays, R, groups, gsizes.
    """
    nloc = n // ncore
    groups = (nloc + 127) // 128
    gsizes = [128] * (groups - 1) + [nloc - 128 * (groups - 1)]

    src = edge_index[0].astype(np.int64)
    dst = edge_index[1].astype(np.int64)
    loop = np.arange(n, dtype=np.int64)
    src = np.concatenate([src, loop])
    dst = np.concatenate([dst, loop])
    deg = np.bincount(dst, minlength=n).astype(np.float32)
    dinv = 1.0 / np.sqrt(np.maximum(deg, 1.0))
    norm = (dinv[src] * dinv[dst]).astype(np.float32)

    # degree-balanced (group, slot) assignment per core
    g_of = np.empty(n, np.int64)
    s_of = np.empty(n, np.int64)
    perm_pos = np.empty(n, np.int64)  # node -> global table row
    node_at = np.empty(n, np.int64)   # global table row -> node
    gseq = _snake_groups(groups, gsizes)
    gbase = np.zeros(groups, np.int64)
    np.cumsum(np.asarray(gsizes[:-1]), out=gbase[1:])
    for c in range(ncore):
        nodes = np.arange(c * nloc, (c + 1) * nloc)
        order = nodes[np.argsort(-deg[nodes], kind="stable")]
        g_assign = gseq  # aligned with `order`
        # slot within group = running count per group
        slot = np.zeros(nloc, np.int64)
        counts = np.zeros(groups, np.int64)
        # vectorized running count per group id
        sort_by_g = np.argsort(g_assign, kind="stable")
        gsorted = g_assign[sort_by_g]
        starts = np.searchsorted(gsorted, np.arange(groups))
        within = np.arange(nloc) - starts[gsorted]
        slot[sort_by_g] = within
        g_of[order] = g_assign
        s_of[order] = slot
        rows = c * nloc + gbase[g_assign] + slot
        perm_pos[order] = rows
        node_at[rows] = order

    # edge -> (core, group) buckets, laid out into blocks of 128
    c_e = dst // nloc
    key = c_e * groups + g_of[dst]
    order_e = np.argsort(key, kind="stable")
    cnt = np.bincount(key, minlength=ncore * groups)
    R = int(np.ceil(cnt.max() / 128))
    B = groups * R
    starts = np.zeros(ncore * groups + 1, np.int64)
    np.cumsum(cnt, out=starts[1:])
    pos_in_bucket = np.arange(len(order_e)) - starts[key[order_e]]
    ks = key[order_e]
    ce = ks // groups
    ge = ks % groups
    flat = ge * (R * 128) + pos_in_bucket  # position inside [B*128]

    gidx = np.zeros((ncore, B * 128), np.int32)
    rel = np.full((ncore, B * 128), -1.0, np.float32)
    nrm = np.zeros((ncore, B * 128), np.float32)
    gidx[ce, flat] = perm_pos[src[order_e]].astype(np.int32)
    rel[ce, flat] = s_of[dst[order_e]].astype(np.float32)
    nrm[ce, flat] = norm[order_e]

    # -> [ncore, 128 partitions, B blocks]
    gidx = gidx.reshape(ncore, B, 128).transpose(0, 2, 1).copy()
    rel = rel.reshape(ncore, B, 128).transpose(0, 2, 1).copy()
    nrm = nrm.reshape(ncore, B, 128).transpose(0, 2, 1).copy()

    return dict(
        gidx=gidx, rel=rel, nrm=nrm, perm_pos=perm_pos, node_at=node_at,
        R=R, groups=groups, gsizes=gsizes, nloc=nloc,
    )


# ---------------------------------------------------------------- bass build

def build_nc(n_nodes, nloc, groups, gsizes, R, n_layers, ncore):
    import concourse.bass as bass
    import concourse.bacc as bacc
    import concourse.mybir as mybir
    import concourse.tile as tile

    f32 = mybir.dt.float32
    bf16 = mybir.dt.bfloat16
    i32 = mybir.dt.int32
    u16 = mybir.dt.uint16
    A = mybir.AluOpType
    AF = mybir.ActivationFunctionType
    B = groups * R

    nc = bacc.Bacc("TRN2", target_bir_lowering=False, num_devices=ncore)
    xl = nc.dram_tensor("xl", [nloc, D], u16, kind="ExternalInput")
    gidx_d = nc.dram_tensor("gidx", [128, B], i32, kind="ExternalInput")
    rel_d = nc.dram_tensor("rel", [128, B], f32, kind="ExternalInput")
    nrm_d = nc.dram_tensor("nrm", [128, B], f32, kind="ExternalInput")
    w_d = nc.dram_tensor("wmat", [D, n_layers * D], bf16, kind="ExternalInput")
    b_d = nc.dram_tensor("bvec", [D, n_layers], f32, kind="ExternalInput")
    iota_d = nc.dram_tensor("iotaf", [128, 128], bf16, kind="ExternalInput")
    idb_d = nc.dram_tensor("identb", [128, 128], bf16, kind="ExternalInput")
    idf_d = nc.dram_tensor("identf", [128, 128], f32, kind="ExternalInput")
    out_d = nc.dram_tensor("out", [nloc, D], f32, kind="ExternalOutput")
    htab = nc.dram_tensor("htab", [n_nodes, D], u16, kind="Internal",
                          addr_space="Shared")
    agin = nc.dram_tensor("agin", [nloc, D], u16, kind="Internal")
    rg = [list(range(ncore))]

    with tile.TileContext(nc) as tc:
        with (
            tc.tile_pool(name="const", bufs=1) as cp,
            tc.tile_pool(name="msg", bufs=8) as mp_,
            tc.tile_pool(name="sp", bufs=3) as spp,
            tc.tile_pool(name="sm", bufs=3) as sm,
            tc.tile_pool(name="ps", bufs=2, space="PSUM") as ps,
            tc.tile_pool(name="ps2", bufs=2, space="PSUM") as ps2,
        ):
            gidx_sb = cp.tile([128, B], i32)
            nc.sync.dma_start(gidx_sb[:], gidx_d.ap())
            w_sb = cp.tile([128, n_layers * D], bf16)
            nc.sync.dma_start(w_sb[:], w_d.ap())
            idb_sb = cp.tile([128, 128], bf16)
            nc.sync.dma_start(idb_sb[:], idb_d.ap())
            idf_sb = cp.tile([128, 128], f32)
            nc.sync.dma_start(idf_sb[:], idf_d.ap())
            # DVE-read constants go through DVE staging copies so the hot
            # tensor_scalar ops (limited sync-wait slots) never carry DMA
            # semaphore waits.
            rel_st = cp.tile([128, B], f32)
            nc.sync.dma_start(rel_st[:], rel_d.ap())
            rel_sb = cp.tile([128, B], f32)
            nc.vector.tensor_copy(rel_sb[:], rel_st[:])
            nrm_st = cp.tile([128, B], f32)
            nc.sync.dma_start(nrm_st[:], nrm_d.ap())
            nrm_sb = cp.tile([128, B], f32)
            nc.vector.tensor_copy(nrm_sb[:], nrm_st[:])
            b_st = cp.tile([128, n_layers], f32)
            nc.sync.dma_start(b_st[:], b_d.ap())
            b_sb = cp.tile([128, n_layers], f32)
            nc.vector.tensor_copy(b_sb[:], b_st[:])
            iota_st = cp.tile([128, 128], bf16)
            nc.sync.dma_start(iota_st[:], iota_d.ap())
            iota_sb = cp.tile([128, 128], bf16)
            nc.vector.tensor_copy(iota_sb[:], iota_st[:])
            jk = cp.tile([128, groups * 128], f32)
            # The qPoolDynamic indirect-DMA encoding has a single sync-wait
            # slot, but a steady-state gather needs two waits (WAW on the
            # recycled msg slot's previous gather + WAR on its PE readers).
            # Dummy 1-wait Pool DMAs ("gates") observe those ticks first so
            # Tile elides the waits on the gather itself.
            gatebuf = cp.tile([1, 1], bf16)

            def pool_gate(dep_inst, reason):
                # A Pool-engine compute no-op (memset, not a DMA: consumes no
                # DMASW lane) that observes `dep_inst`'s completion tick on
                # the Pool clock, so the next gather's matching wait is
                # elided. Gate-to-gate WAW deps are same-engine program
                # order, so sharing one slot costs no semaphores.
                from concourse.tile_rust import add_dep_helper
                gi = nc.gpsimd.memset(gatebuf[0:1, 0:1], 0.0)
                add_dep_helper(gi.ins, dep_inst, sync=True, reason=reason)
                return gi.ins

            # msg bufs == number of DMASW lanes: since gathers are the ONLY
            # Pool DMAs, gather q's WAW predecessor (gather q-8) sits on the
            # same lane as its own lane-recycle wait -> one combined wait,
            # which is all the indirect-DMA encoding can carry.
            MSG_BUFS = 8
            hist = []  # (gather_inst, last_matmul_inst) per group, in order

            # table #0 = allgathered x
            nc.sync.dma_start(agin.ap(), xl.ap())
            cc0 = nc.gpsimd.collective_compute(
                "AllGather", A.bypass, replica_groups=rg,
                ins=[agin.ap().opt()], outs=[htab.ap().opt()],
            )
            pool_gate(cc0.ins, "observe allgather")

            for l in range(n_layers):
                last = l == n_layers - 1
                for g in range(groups):
                    gsz = gsizes[g]
                    colsl = slice(g * R, (g + 1) * R)
                    from concourse.tile_rust import add_dep_helper
                    msg = mp_.tile([128, R * 128], u16, tag="msg")
                    pregates = []
                    q = len(hist)
                    if q >= MSG_BUFS:
                        omm = hist[q - MSG_BUFS]
                        pregates.append(pool_gate(omm, "msg WAR observe"))
                    for r in range(R):
                        b = g * R + r
                        gat = nc.gpsimd.indirect_dma_start(
                            out=msg[:, r * 128:(r + 1) * 128], out_offset=None,
                            in_=htab.ap(),
                            in_offset=bass.IndirectOffsetOnAxis(
                                ap=gidx_sb[:, b:b + 1], axis=0),
                        )
                        for pg in pregates:
                            add_dep_helper(gat.ins, pg, sync=False,
                                           reason="gate before gather")
                    sp = spp.tile([128, R * 128], bf16, tag="sp")
                    for r in range(R):
                        b = g * R + r
                        nc.vector.tensor_scalar(
                            sp[:, r * 128:(r + 1) * 128], iota_sb[:],
                            rel_sb[:, b:b + 1], nrm_sb[:, b:b + 1],
                            op0=A.is_equal, op1=A.mult)
                    agg = ps.tile([128, 128], f32, tag="agg")
                    for r in range(R):
                        bs = slice(r * 128, (r + 1) * 128)
                        mm = nc.tensor.matmul(
                            agg[:], lhsT=msg[:, bs].bitcast(bf16),
                            rhs=sp[:, bs],
                            start=(r == 0), stop=(r == R - 1))
                    hist.append(mm.ins)
                    aggs = sm.tile([128, 128], bf16, tag="aggs")
                    nc.vector.tensor_copy(aggs[:], agg[:])
                    z = ps.tile([128, 128], f32, tag="z")
                    nc.tensor.matmul(
                        z[:], lhsT=w_sb[:, l * D:(l + 1) * D], rhs=aggs[:],
                        start=True, stop=True)
                    bap = b_sb[:, l:l + 1]
                    hT = sm.tile([128, 128], f32, tag="hT")
                    if not last:
                        zmin = sm.tile([128, 128], f32, tag="zmin")
                        nc.vector.tensor_scalar(
                            zmin[:], z[:], bap, 0.0, op0=A.add, op1=A.min)
                        ex = sm.tile([128, 128], f32, tag="ex")
                        nc.scalar.activation(ex[:], zmin[:], AF.Exp)
                        relp = sm.tile([128, 128], f32, tag="relp")
                        nc.vector.tensor_scalar(
                            relp[:], z[:], bap, 0.0, op0=A.add, op1=A.max)
                        t0 = sm.tile([128, 128], f32, tag="t0")
                        nc.vector.tensor_tensor(
                            out=t0[:], in0=relp[:], in1=ex[:], op=A.add)
                        nc.vector.tensor_scalar(
                            hT[:], t0[:], -1.0, None, op0=A.add)
                    else:
                        nc.vector.tensor_scalar(
                            hT[:], z[:], bap, None, op0=A.add)
                    jslice = jk[:, g * 128:(g + 1) * 128]
                    if l == 0:
                        nc.vector.tensor_copy(jslice, hT[:])
                    else:
                        nc.vector.tensor_tensor(
                            out=jslice, in0=jslice, in1=hT[:], op=A.max)
                    if not last:
                        hTb = sm.tile([128, 128], bf16, tag="hTb")
                        nc.scalar.activation(hTb[:], hT[:], AF.Copy)
                        hp = ps2.tile([128, 128], bf16, tag="hp")
                        nc.tensor.transpose(hp[:], hTb[:], idb_sb[:])
                        hsb = sm.tile([128, 128], bf16, tag="hsb")
                        nc.vector.tensor_copy(hsb[:], hp[:])
                        base = 128 * g
                        nc.sync.dma_start(
                            agin.ap()[base:base + gsz, :],
                            hsb[:gsz, :].bitcast(u16))
                if not last:
                    ccl = nc.gpsimd.collective_compute(
                        "AllGather", A.bypass, replica_groups=rg,
                        ins=[agin.ap().opt()], outs=[htab.ap().opt()],
                    )
                    pool_gate(ccl.ins, "observe allgather")

            for g in range(groups):
                gsz = gsizes[g]
                op_ = ps2.tile([128, 128], f32, tag="op")
                nc.tensor.transpose(
                    op_[:], jk[:, g * 128:(g + 1) * 128], idf_sb[:])
                osb = sm.tile([128, 128], f32, tag="osb")
                nc.vector.tensor_copy(osb[:], op_[:])
                base = 128 * g
                nc.sync.dma_start(out_d.ap()[base:base + gsz, :], osb[:gsz, :])
    nc.compile()
    return nc


# ---------------------------------------------------------------- entry

def _prepare_inputs(x, edge_index, W0, b0, Ws, bs, n, ncore, n_layers):
    import ml_dtypes

    bf16 = ml_dtypes.bfloat16
    gd = prepare_graph(np.asarray(edge_index), n, ncore)
    nloc, groups, R = gd["nloc"], gd["groups"], gd["R"]

    x = np.asarray(x, np.float32)
    Wall = np.concatenate(
        [np.asarray(W0, np.float32)[None]] + [np.asarray(Ws, np.float32)],
        axis=0)  # [L, D, D]
    ball = np.concatenate(
        [np.asarray(b0, np.float32)[None]] + [np.asarray(bs, np.float32)],
        axis=0)  # [L, D]
    wmat = np.ascontiguousarray(
        Wall.transpose(1, 0, 2).reshape(D, n_layers * D)).astype(bf16)
    bvec = np.ascontiguousarray(ball.T).astype(np.float32)  # [D, L]
    iotaf = np.broadcast_to(
        np.arange(128, dtype=np.float32), (128, 128)).astype(bf16)
    identb = np.eye(128, dtype=np.float32).astype(bf16)
    identf = np.eye(128, dtype=np.float32)

    xp = x.astype(bf16)[gd["node_at"]]  # permuted into table order
    in_maps = []
    for c in range(ncore):
        in_maps.append({
            "xl": np.ascontiguousarray(
                xp[c * nloc:(c + 1) * nloc]).view(np.uint16),
            "gidx": gd["gidx"][c],
            "rel": gd["rel"][c],
            "nrm": gd["nrm"][c],
            "wmat": wmat,
            "bvec": bvec,
            "iotaf": np.ascontiguousarray(iotaf),
            "identb": identb,
            "identf": identf,
        })
    return in_maps, gd


LAST_RESULTS = None


def kernel(x, edge_index, W0, b0, Ws, bs):
    global LAST_RESULTS
    from concourse import bass_utils

    n = np.asarray(x).shape[0]
    in_maps, gd = _prepare_inputs(
        x, edge_index, W0, b0, Ws, bs, n, N_CORES, K_LAYERS)
    nc = build_nc(n, gd["nloc"], gd["groups"], gd["gsizes"], gd["R"],
                  K_LAYERS, N_CORES)
    res = bass_utils.run_bass_kernel_spmd(
        nc, in_maps, core_ids=list(range(N_CORES)))
    LAST_RESULTS = res
    out_perm = np.concatenate([r["out"] for r in res.results], axis=0)
    return np.ascontiguousarray(out_perm[gd["perm_pos"]]).astype(np.float32)
